# revision 34
# baseline (speedup 1.0000x reference)
"""Multi-head attention Trainium2 kernel (8 NeuronCores).

Sharding: data-parallel over batch (4 pairs of cores) x tensor-parallel over
heads (2-way split within each pair). Core c handles batch c//2 and heads
(c%2)*8 .. (c%2)*8+8. The output projection uses row-parallel Wo with an
on-device ReduceScatter within each core pair; each core emits half the
sequence rows of its batch, and the host only concatenates.

Math notes vs. the reference:
 - reference subtracts the row max (over ALL keys, pre-mask) inside exp and
   adds EPS=1e-7 to the softmax denominator. Since scores = q.k/8 >= 0
   (q,k are post-relu) and bounded (~<6), exp never overflows without the
   max subtraction, every row's denominator is >= 1, and both the max
   subtraction and EPS cancel to < 1e-5 relative. So we compute
   a = exp(s/8)*causal / sum(exp(s/8)*causal) directly.
"""

import numpy as np
import ml_dtypes

B, S, D, H = 4, 2048, 1024, 16
HD = 64          # head dim
HC = 8           # heads per core
DC = HC * HD     # 512 head-dims per core
NCORES = 8

_cache = {}


def _build(use_collective=True):
    import concourse.bass as bass
    import concourse.mybir as mybir
    import concourse.tile as tile
    from concourse import bacc
    from concourse.masks import make_upper_triangular

    f32 = mybir.dt.float32
    f32r = mybir.dt.float32r
    bf16 = mybir.dt.bfloat16
    AF = mybir.ActivationFunctionType

    nc = bacc.Bacc("TRN2", target_bir_lowering=False, debug=False,
                   num_devices=NCORES)

    xT_d = nc.dram_tensor("xT", [D, S], bf16, kind="ExternalInput")
    wq_d = nc.dram_tensor("wq", [D, DC], bf16, kind="ExternalInput")
    wk_d = nc.dram_tensor("wk", [D, DC], bf16, kind="ExternalInput")
    wv_d = nc.dram_tensor("wv", [D, DC], bf16, kind="ExternalInput")
    wo_d = nc.dram_tensor("wo", [DC, D], f32r, kind="ExternalInput")
    bq_d = nc.dram_tensor("bq", [128, 4], f32, kind="ExternalInput")
    bk_d = nc.dram_tensor("bk", [128, 4], f32, kind="ExternalInput")
    bvb_d = nc.dram_tensor("bvb", [128, DC], f32, kind="ExternalInput")
    bob_d = nc.dram_tensor("bob", [128, D], f32, kind="ExternalInput")
    y_d = nc.dram_tensor("y", [S // 2, D], f32, kind="ExternalOutput")

    NQT = S // 512          # 4 q-tiles of 512
    NKB = S // 128          # 16 k-blocks of 128
    NST = S // 128          # 16 s-tiles for v
    NCH = D // 128          # 8 contraction chunks for projections

    def r(ap):
        return ap.bitcast(f32r)

    lowp = nc.allow_low_precision("fp32r matmul inputs")
    lowp.__enter__()
    with tile.TileContext(nc) as tc:
        with (
            tc.tile_pool(name="const", bufs=1) as cp,
            tc.tile_pool(name="xt", bufs=1) as xp,
            tc.tile_pool(name="proj", bufs=1) as pp,
            tc.tile_pool(name="ework", bufs=4) as ep,
            tc.tile_pool(name="small", bufs=2) as sp,
            tc.tile_pool(name="evac", bufs=3) as vp,
            tc.tile_pool(name="ps", bufs=2, space="PSUM") as psp,
            tc.tile_pool(name="ctxps", bufs=2, space="PSUM") as cxp,
            tc.tile_pool(name="opps", bufs=2, space="PSUM") as opp,
            tc.tile_pool(name="dram", bufs=1, space="DRAM") as dp,
        ):
            # ---- constants ----
            tri = cp.tile([128, 128], bf16, name="tri", tag="tri")
            make_upper_triangular(nc, tri[:], val=1.0, diag=True)
            ones_f = cp.tile([128, 64], bf16, name="ones_f", tag="ones_f")
            nc.vector.memset(ones_f[:], 1.0)
            ones64 = cp.tile([1, 64], f32r, name="ones64", tag="ones64")
            nc.vector.tensor_copy(ones64[:], ones_f[0:1, :])
            bq_t = cp.tile([128, 4], f32, name="bq", tag="bq")
            nc.gpsimd.dma_start(bq_t[:], bq_d[:])
            bk_t = cp.tile([128, 4], f32, name="bk", tag="bk")
            nc.gpsimd.dma_start(bk_t[:], bk_d[:])
            bvb_t = cp.tile([128, DC], f32, name="bvb", tag="bvb")
            nc.gpsimd.dma_start(bvb_t[:], bvb_d[:])
            bob_t = cp.tile([128, D], f32, name="bob", tag="bob")
            nc.gpsimd.dma_start(bob_t[:], bob_d[:])

            # ---- x^T resident: tiles [128, 512] per (chunk, seg) ----
            # load seg 0 first, then wq (so the first qT matmul isn't
            # queued behind the remaining x segments), then segs 1-3
            xt = [[None] * 4 for _ in range(NCH)]
            wq_pre = []
            for sg in range(4):
                for c in range(NCH):
                    t = xp.tile([128, 512], bf16, name=f"xt{c}_{sg}",
                                tag=f"xt{c}_{sg}")
                    eng = nc.sync if c % 2 == 0 else nc.scalar
                    eng.dma_start(
                        t[:], xT_d[c * 128:(c + 1) * 128,
                                   sg * 512:(sg + 1) * 512])
                    xt[c][sg] = t
                if sg == 0:
                    for c in range(NCH):
                        wt = pp.tile([128, DC], bf16, name=f"wq{c}",
                                     tag=f"w{c}")
                        eng = nc.scalar if c % 2 == 0 else nc.sync
                        eng.dma_start(
                            wt[:], wq_d[c * 128:(c + 1) * 128, :])
                        wq_pre.append(wt)

            # ---- q^T and k^T projections: [DC, S] as 4 tiles [128, S] ----
            # tile t holds local heads 2t (partitions 0:64) and 2t+1 (64:128)
            qT, kT = [], []
            for (w_d, bias_t, out_list, nm) in (
                (wq_d, bq_t, qT, "q"), (wk_d, bk_t, kT, "k"),
            ):
                if nm == "q":
                    wch = wq_pre
                else:
                    wch = []
                    for c in range(NCH):
                        wt = pp.tile([128, DC], bf16, name=f"w{nm}{c}",
                                     tag=f"w{c}")
                        eng = nc.scalar if c % 2 == 0 else nc.sync
                        eng.dma_start(wt[:], w_d[c * 128:(c + 1) * 128, :])
                        wch.append(wt)
                for t in range(4):
                    out = pp.tile([128, S], bf16, name=f"{nm}T{t}", tag=f"{nm}T{t}")
                    out_list.append(out)
                    for seg in range(4):
                        ps = psp.tile([128, 1024], f32, name="ps",
                                      tag="sc", bufs=2)
                        for c in range(NCH):
                            nc.tensor.matmul(
                                ps[:, 0:512],
                                wch[c][:, t * 128:(t + 1) * 128],
                                xt[c][seg][:],
                                start=(c == 0), stop=(c == NCH - 1),
                            )
                        nc.vector.tensor_scalar(
                            out[:, seg * 512:(seg + 1) * 512], ps[:, 0:512],
                            bias_t[:, t:t + 1], 0.0,
                            mybir.AluOpType.add, mybir.AluOpType.max,
                        )

            # ---- v projection into augmented layout [128, HC, 65] ----
            # per s-tile: columns h*65..h*65+63 are relu(x@wv+bv) for local
            # head h; column h*65+64 is 1.0 (for softmax row sums).
            wvch = []
            for c in range(NCH):
                wt = pp.tile([128, DC], bf16, name=f"wv{c}", tag=f"w{c}")
                nc.sync.dma_start(wt[:], wv_d[c * 128:(c + 1) * 128, :])
                wvch.append(wt)
            vav = []
            for st in range(NST):
                va = pp.tile([128, HC, 65], bf16, name=f"va{st}", tag=f"va{st}")
                vav.append(va)
                ps = psp.tile([128, 1024], f32, name="ps", tag="sc", bufs=2)
                for c in range(NCH):
                    nc.tensor.matmul(
                        ps[:, 0:512],
                        xt[c][st // 4][:, (st % 4) * 128:(st % 4) * 128 + 128],
                        wvch[c][:],
                        start=(c == 0), stop=(c == NCH - 1),
                    )
                nc.vector.tensor_add(ps[:, 0:512], ps[:, 0:512], bvb_t[:])
                nc.vector.tensor_scalar(
                    va[:, :, 0:64],
                    ps[:, 0:512].rearrange("p (h d) -> p h d", h=HC),
                    0.0, None, mybir.AluOpType.max,
                )
                nc.vector.tensor_copy(
                    va[:, :, 64:65],
                    ones_f[:, 0:8].rearrange("p (h o) -> p h o", o=1))

            # ---- attention (j outer, pairs inner) + per-chunk o-proj/CC ----
            ctxT = [pp.tile([128, S], f32r, name=f"cxt{t}", tag=f"cxt{t}")
                    for t in range(4)]
            rs_in = dp.tile([S, D], f32, name="rsin", tag="rsin")
            rs_out = dp.tile([S // 2, D], f32, name="rsout", tag="rsout")
            woch = {}
            for c in range(4):
                for half in range(2):
                    wt = pp.tile([128, 512], f32r, name=f"woc{c}h{half}",
                                 tag=f"w{c * 2 + half}")
                    nc.sync.dma_start(
                        wt[:], wo_d[c * 128:(c + 1) * 128,
                                    half * 512:(half + 1) * 512])
                    woch[(c, half)] = wt
            for j in reversed(range(NQT)):
                for p in range(4):
                    ctxA = cxp.tile([65, 512], f32, name="ctx", tag="ctx", bufs=2)
                    ctxB = cxp.tile([65, 512], f32, name="ctx", tag="ctx", bufs=2)
                    nblk = 4 * j + 4
                    for kb in range(nblk):
                        dlt = kb * 128 - j * 512
                        qoff = max(dlt, 0)
                        w = 512 - qoff
                        qlo = j * 512 + qoff
                        sc = psp.tile([128, 1024], f32, name="sc",
                                      tag="sc", bufs=2)
                        e = ep.tile([128, 1024], bf16, name="e", tag="e", bufs=3)
                        for (hh, tpos) in ((0, (0, 0)), (1, (64, 0))):
                            plo = hh * 64
                            nc.tensor.matmul(
                                sc[:, hh * 512:hh * 512 + w],
                                kT[p][plo:plo + 64,
                                      kb * 128:(kb + 1) * 128],
                                qT[p][plo:plo + 64, qlo:qlo + w],
                                start=True, stop=True, tile_position=tpos,
                            )
                        nc.scalar.activation(
                            e[:].rearrange("p (h q) -> p h q", h=2)[:, :, 0:w],
                            sc[:].rearrange("p (h q) -> p h q", h=2)[:, :, 0:w],
                            AF.Exp, bias=0.0, scale=0.125,
                        )
                        if dlt >= 0:
                            nc.vector.tensor_mul(
                                e[:, 0:128], e[:, 0:128], tri[:])
                            nc.vector.tensor_mul(
                                e[:, 512:640], e[:, 512:640], tri[:])
                        for (hh, ctx) in ((0, ctxA), (1, ctxB)):
                            nc.tensor.matmul(
                                ctx[:, qoff:qoff + w],
                                vav[kb][:, 2 * p + hh, :],
                                e[:, hh * 512:hh * 512 + w],
                                start=(kb == 0), stop=(kb == nblk - 1),
                                skip_group_check=True,
                            )
                    # evacuate ctx psum fast (frees bank), normalize in SBUF
                    for (hh, ctx) in ((0, ctxA), (1, ctxB)):
                        cu = sp.tile([64, 512], f32r, name="cu", tag="cu",
                                     bufs=2)
                        nc.vector.tensor_copy(cu[:], ctx[0:64, :])
                        rho = sp.tile([1, 512], f32, name="rho", tag="rho",
                                      bufs=4)
                        nc.vector.tensor_copy(rho[:], ctx[64:65, :])
                        rc1 = sp.tile([1, 512], f32, name="rc1", tag="rc1",
                                      bufs=4)
                        nc.vector.reciprocal_approx_fast(rc1[:], rho[:])
                        rcp = sp.tile([64, 512], f32, name="rcp", tag="rcp")
                        nc.gpsimd.partition_broadcast(rcp[:], rc1[:])
                        nc.vector.tensor_mul(
                            ctxT[p][hh * 64:hh * 64 + 64,
                                    j * 512:(j + 1) * 512],
                            cu[:], rcp[:])

                # ---- o-proj + reduce-scatter for q-chunk(s) of j ----
                subchunks = [(j * 512, 512)] if j > 0 else [(0, 256),
                                                           (256, 256)]
                for (cs, cn) in subchunks:
                  for qt in range(cs // 128, (cs + cn) // 128):
                    ys = vp.tile([128, D], f32, name="ys", tag="ys",
                                 bufs=2)
                    for half in range(2):
                        op = opp.tile([128, 512], f32, name="op", tag="op",
                                      bufs=2)
                        for c in range(4):
                            nc.tensor.matmul(
                                op[:],
                                ctxT[c][:, qt * 128:(qt + 1) * 128],
                                woch[(c, half)][:],
                                start=(c == 0), stop=(c == 3),
                            )
                        nc.vector.tensor_copy(
                            ys[:, half * 512:(half + 1) * 512], op[:])
                    nc.sync.dma_start(rs_in[qt * 128:(qt + 1) * 128, :],
                                        ys[:])
                  if use_collective:
                    nc.gpsimd.collective_compute(
                        "ReduceScatter",
                        bass.mybir.AluOpType.add,
                        replica_groups=[[0, 1], [2, 3], [4, 5], [6, 7]],
                        ins=[rs_in[cs:cs + cn, :].opt()],
                        outs=[rs_out[cs // 2:(cs + cn) // 2, :].opt()],
                    )
                  else:
                    nc.sync.dma_start(rs_out[cs // 2:(cs + cn) // 2, :],
                                      rs_in[cs:cs + cn // 2, :])


            # ---- deferred tail: bias + relu on reduce-scattered shards ----
            tail_rows = [768, 896, 512, 640, 256, 384, 0, 128]
            for r0 in tail_rows:
                yt = vp.tile([128, D], f32, name="yt", tag="yt", bufs=2)
                nc.sync.dma_start(yt[:], rs_out[r0:r0 + 128, :])
                yo = vp.tile([128, D], f32, name="yo", tag="yo", bufs=2)
                nc.vector.tensor_add(yo[:], yt[:], bob_t[:])
                nc.scalar.activation(yo[:], yo[:], AF.Relu,
                                     bias=0.0, scale=1.0)
                nc.sync.dma_start(y_d[r0:r0 + 128, :], yo[:])

    lowp.__exit__(None, None, None)
    nc.compile()
    return nc


def _get_nc():
    if "nc" not in _cache:
        _cache["nc"] = _build()
    return _cache["nc"]


def kernel(x, Wq, bq, Wk, bk, Wv, bv, Wo, bo, trace=False):
    from concourse.bass_utils import run_bass_kernel_spmd

    x = np.asarray(x, np.float32)
    Wq, bq = np.asarray(Wq, np.float32), np.asarray(bq, np.float32)
    Wk, bk = np.asarray(Wk, np.float32), np.asarray(bk, np.float32)
    Wv, bv = np.asarray(Wv, np.float32), np.asarray(bv, np.float32)
    Wo, bo = np.asarray(Wo, np.float32), np.asarray(bo, np.float32)

    nc = _get_nc()
    in_maps = []
    for c in range(NCORES):
        b, hh = c // 2, c % 2
        sl = slice(hh * DC, (hh + 1) * DC)
        in_maps.append({
            "xT": np.ascontiguousarray(x[b].T).astype(ml_dtypes.bfloat16),
            "wq": np.ascontiguousarray(Wq[:, sl]).astype(ml_dtypes.bfloat16),
            "wk": np.ascontiguousarray(Wk[:, sl]).astype(ml_dtypes.bfloat16),
            "wv": np.ascontiguousarray(Wv[:, sl]).astype(ml_dtypes.bfloat16),
            "wo": np.ascontiguousarray(Wo[sl, :]),
            "bq": np.ascontiguousarray(bq[sl].reshape(4, 128).T),
            "bk": np.ascontiguousarray(bk[sl].reshape(4, 128).T),
            "bvb": np.ascontiguousarray(
                np.broadcast_to(bv[sl], (128, DC))),
            "bob": np.ascontiguousarray(np.broadcast_to(bo, (128, D))),
        })

    res = run_bass_kernel_spmd(nc, in_maps, core_ids=list(range(NCORES)),
                               trace=trace)
    _cache["last_result"] = res

    y = np.empty((B, S, D), np.float32)
    chunks = [(1536, 512), (1024, 512), (512, 512), (0, 256), (256, 256)]
    for c in range(NCORES):
        b, hh = c // 2, c % 2
        yp = res.results[c]["y"]
        for (s, n) in chunks:
            h = n // 2
            y[b, s + hh * h:s + hh * h + h, :] = yp[s // 2:s // 2 + h]
    return y


# revision 35
# speedup vs baseline: 1.0930x; 1.0930x over previous
"""Multi-head attention Trainium2 kernel (8 NeuronCores).

Sharding: data-parallel over batch (4 pairs of cores) x tensor-parallel over
heads (2-way split within each pair). Core c handles batch c//2 and heads
(c%2)*8 .. (c%2)*8+8. The output projection uses row-parallel Wo with an
on-device ReduceScatter within each core pair; each core emits half the
sequence rows of its batch, and the host only concatenates.

Math notes vs. the reference:
 - reference subtracts the row max (over ALL keys, pre-mask) inside exp and
   adds EPS=1e-7 to the softmax denominator. Since scores = q.k/8 >= 0
   (q,k are post-relu) and bounded (~<6), exp never overflows without the
   max subtraction, every row's denominator is >= 1, and both the max
   subtraction and EPS cancel to < 1e-5 relative. So we compute
   a = exp(s/8)*causal / sum(exp(s/8)*causal) directly.
"""

import numpy as np
import ml_dtypes

B, S, D, H = 4, 2048, 1024, 16
HD = 64          # head dim
HC = 8           # heads per core
DC = HC * HD     # 512 head-dims per core
NCORES = 8

_cache = {}


def _build(use_collective=True):
    import concourse.bass as bass
    import concourse.mybir as mybir
    import concourse.tile as tile
    from concourse import bacc
    from concourse.masks import make_upper_triangular

    f32 = mybir.dt.float32
    f32r = mybir.dt.float32r
    bf16 = mybir.dt.bfloat16
    AF = mybir.ActivationFunctionType

    nc = bacc.Bacc("TRN2", target_bir_lowering=False, debug=False,
                   num_devices=NCORES)

    xT_d = nc.dram_tensor("xT", [D, S], bf16, kind="ExternalInput")
    wq_d = nc.dram_tensor("wq", [D, DC], bf16, kind="ExternalInput")
    wk_d = nc.dram_tensor("wk", [D, DC], bf16, kind="ExternalInput")
    wv_d = nc.dram_tensor("wv", [D, DC], bf16, kind="ExternalInput")
    wo_d = nc.dram_tensor("wo", [DC, D], f32r, kind="ExternalInput")
    bq_d = nc.dram_tensor("bq", [128, 4], f32, kind="ExternalInput")
    bk_d = nc.dram_tensor("bk", [128, 4], f32, kind="ExternalInput")
    bvb_d = nc.dram_tensor("bvb", [128, DC], f32, kind="ExternalInput")
    bob_d = nc.dram_tensor("bob", [128, D], f32, kind="ExternalInput")
    y_d = nc.dram_tensor("y", [S // 2, D], f32, kind="ExternalOutput")

    NQT = S // 512          # 4 q-tiles of 512
    NKB = S // 128          # 16 k-blocks of 128
    NST = S // 128          # 16 s-tiles for v
    NCH = D // 128          # 8 contraction chunks for projections

    def r(ap):
        return ap.bitcast(f32r)

    lowp = nc.allow_low_precision("fp32r matmul inputs")
    lowp.__enter__()
    with tile.TileContext(nc) as tc:
        with (
            tc.tile_pool(name="const", bufs=1) as cp,
            tc.tile_pool(name="xt", bufs=1) as xp,
            tc.tile_pool(name="proj", bufs=1) as pp,
            tc.tile_pool(name="ework", bufs=4) as ep,
            tc.tile_pool(name="small", bufs=2) as sp,
            tc.tile_pool(name="evac", bufs=3) as vp,
            tc.tile_pool(name="ps", bufs=2, space="PSUM") as psp,
            tc.tile_pool(name="ctxps", bufs=2, space="PSUM") as cxp,
            tc.tile_pool(name="opps", bufs=2, space="PSUM") as opp,
            tc.tile_pool(name="dram", bufs=1, space="DRAM") as dp,
        ):
            # ---- constants ----
            tri = cp.tile([128, 128], bf16, name="tri", tag="tri")
            make_upper_triangular(nc, tri[:], val=1.0, diag=True)
            ones_f = cp.tile([128, 64], bf16, name="ones_f", tag="ones_f")
            nc.vector.memset(ones_f[:], 1.0)
            ones64 = cp.tile([1, 64], f32r, name="ones64", tag="ones64")
            nc.vector.tensor_copy(ones64[:], ones_f[0:1, :])
            bq_t = cp.tile([128, 4], f32, name="bq", tag="bq")
            nc.gpsimd.dma_start(bq_t[:], bq_d[:])
            bk_t = cp.tile([128, 4], f32, name="bk", tag="bk")
            nc.gpsimd.dma_start(bk_t[:], bk_d[:])
            bvb_t = cp.tile([128, DC], f32, name="bvb", tag="bvb")
            nc.gpsimd.dma_start(bvb_t[:], bvb_d[:])
            bob_t = cp.tile([128, D], f32, name="bob", tag="bob")
            nc.gpsimd.dma_start(bob_t[:], bob_d[:])

            # ---- x^T resident: tiles [128, 512] per (chunk, seg) ----
            # load seg 0 first, then wq (so the first qT matmul isn't
            # queued behind the remaining x segments), then segs 1-3
            xt = [[None] * 4 for _ in range(NCH)]
            wq_pre = []
            for sg in range(4):
                for c in range(NCH):
                    t = xp.tile([128, 512], bf16, name=f"xt{c}_{sg}",
                                tag=f"xt{c}_{sg}")
                    eng = nc.sync if c % 2 == 0 else nc.scalar
                    eng.dma_start(
                        t[:], xT_d[c * 128:(c + 1) * 128,
                                   sg * 512:(sg + 1) * 512])
                    xt[c][sg] = t
                if sg == 0:
                    for c in range(NCH):
                        wt = pp.tile([128, DC], bf16, name=f"wq{c}",
                                     tag=f"w{c}")
                        eng = nc.scalar if c % 2 == 0 else nc.sync
                        eng.dma_start(
                            wt[:], wq_d[c * 128:(c + 1) * 128, :])
                        wq_pre.append(wt)

            # ---- q^T and k^T projections: [DC, S] as 4 tiles [128, S] ----
            # tile t holds local heads 2t (partitions 0:64) and 2t+1 (64:128)
            qT, kT = [], []
            for (w_d, bias_t, out_list, nm) in (
                (wq_d, bq_t, qT, "q"), (wk_d, bk_t, kT, "k"),
            ):
                if nm == "q":
                    wch = wq_pre
                else:
                    wch = []
                    for c in range(NCH):
                        wt = pp.tile([128, DC], bf16, name=f"w{nm}{c}",
                                     tag=f"w{c}")
                        eng = nc.scalar if c % 2 == 0 else nc.sync
                        eng.dma_start(wt[:], w_d[c * 128:(c + 1) * 128, :])
                        wch.append(wt)
                for t in range(4):
                    out = pp.tile([128, S], bf16, name=f"{nm}T{t}", tag=f"{nm}T{t}")
                    out_list.append(out)
                    for seg in range(4):
                        ps = psp.tile([128, 1024], f32, name="ps",
                                      tag="sc", bufs=2)
                        for c in range(NCH):
                            nc.tensor.matmul(
                                ps[:, 0:512],
                                wch[c][:, t * 128:(t + 1) * 128],
                                xt[c][seg][:],
                                start=(c == 0), stop=(c == NCH - 1),
                            )
                        nc.vector.tensor_scalar(
                            out[:, seg * 512:(seg + 1) * 512], ps[:, 0:512],
                            bias_t[:, t:t + 1], 0.0,
                            mybir.AluOpType.add, mybir.AluOpType.max,
                        )

            # ---- v projection into augmented layout [128, HC, 65] ----
            # per s-tile: columns h*65..h*65+63 are relu(x@wv+bv) for local
            # head h; column h*65+64 is 1.0 (for softmax row sums).
            wvch = []
            for c in range(NCH):
                wt = pp.tile([128, DC], bf16, name=f"wv{c}", tag=f"w{c}")
                nc.sync.dma_start(wt[:], wv_d[c * 128:(c + 1) * 128, :])
                wvch.append(wt)
            vav = []
            for st in range(NST):
                va = pp.tile([128, HC, 65], bf16, name=f"va{st}", tag=f"va{st}")
                vav.append(va)
                ps = psp.tile([128, 1024], f32, name="ps", tag="sc", bufs=2)
                for c in range(NCH):
                    nc.tensor.matmul(
                        ps[:, 0:512],
                        xt[c][st // 4][:, (st % 4) * 128:(st % 4) * 128 + 128],
                        wvch[c][:],
                        start=(c == 0), stop=(c == NCH - 1),
                    )
                nc.vector.tensor_add(ps[:, 0:512], ps[:, 0:512], bvb_t[:])
                nc.vector.tensor_scalar(
                    va[:, :, 0:64],
                    ps[:, 0:512].rearrange("p (h d) -> p h d", h=HC),
                    0.0, None, mybir.AluOpType.max,
                )
                nc.vector.tensor_copy(
                    va[:, :, 64:65],
                    ones_f[:, 0:8].rearrange("p (h o) -> p h o", o=1))

            # ---- attention (j outer, pairs inner) + per-chunk o-proj/CC ----
            ctxT = [pp.tile([128, S], f32r, name=f"cxt{t}", tag=f"cxt{t}")
                    for t in range(4)]
            rs_in = dp.tile([S, D], f32, name="rsin", tag="rsin")
            rs_out = dp.tile([S // 2, D], f32, name="rsout", tag="rsout")
            woch = {}
            for c in range(4):
                for half in range(2):
                    wt = pp.tile([128, 512], f32r, name=f"woc{c}h{half}",
                                 tag=f"w{c * 2 + half}")
                    nc.sync.dma_start(
                        wt[:], wo_d[c * 128:(c + 1) * 128,
                                    half * 512:(half + 1) * 512])
                    woch[(c, half)] = wt
            for j in reversed(range(NQT)):
                for p in range(4):
                    ctxA = cxp.tile([65, 512], f32, name="ctx", tag="ctx", bufs=2)
                    ctxB = cxp.tile([65, 512], f32, name="ctx", tag="ctx", bufs=2)
                    nblk = 4 * j + 4
                    for kb in range(nblk):
                        dlt = kb * 128 - j * 512
                        qoff = max(dlt, 0)
                        w = 512 - qoff
                        qlo = j * 512 + qoff
                        sc = psp.tile([128, 1024], f32, name="sc",
                                      tag="sc", bufs=2)
                        e = ep.tile([128, 1024], bf16, name="e", tag="e", bufs=6)
                        for (hh, tpos) in ((0, (0, 0)), (1, (64, 0))):
                            plo = hh * 64
                            nc.tensor.matmul(
                                sc[:, hh * 512:hh * 512 + w],
                                kT[p][plo:plo + 64,
                                      kb * 128:(kb + 1) * 128],
                                qT[p][plo:plo + 64, qlo:qlo + w],
                                start=True, stop=True, tile_position=tpos,
                            )
                        nc.scalar.activation(
                            e[:].rearrange("p (h q) -> p h q", h=2)[:, :, 0:w],
                            sc[:].rearrange("p (h q) -> p h q", h=2)[:, :, 0:w],
                            AF.Exp, bias=0.0, scale=0.125,
                        )
                        if dlt >= 0:
                            nc.vector.tensor_mul(
                                e[:, 0:128], e[:, 0:128], tri[:])
                            nc.vector.tensor_mul(
                                e[:, 512:640], e[:, 512:640], tri[:])
                        for (hh, ctx) in ((0, ctxA), (1, ctxB)):
                            nc.tensor.matmul(
                                ctx[:, qoff:qoff + w],
                                vav[kb][:, 2 * p + hh, :],
                                e[:, hh * 512:hh * 512 + w],
                                start=(kb == 0), stop=(kb == nblk - 1),
                                skip_group_check=True,
                            )
                    # evacuate ctx psum fast (frees bank), normalize in SBUF
                    for (hh, ctx) in ((0, ctxA), (1, ctxB)):
                        cu = sp.tile([64, 512], f32r, name="cu", tag="cu",
                                     bufs=4)
                        nc.vector.tensor_copy(cu[:], ctx[0:64, :])
                        rho = sp.tile([1, 512], f32, name="rho", tag="rho",
                                      bufs=4)
                        nc.vector.tensor_copy(rho[:], ctx[64:65, :])
                        rc1 = sp.tile([1, 512], f32, name="rc1", tag="rc1",
                                      bufs=4)
                        nc.vector.reciprocal_approx_fast(rc1[:], rho[:])
                        rcp = sp.tile([64, 512], f32, name="rcp", tag="rcp", bufs=4)
                        nc.gpsimd.partition_broadcast(rcp[:], rc1[:])
                        nc.vector.tensor_mul(
                            ctxT[p][hh * 64:hh * 64 + 64,
                                    j * 512:(j + 1) * 512],
                            cu[:], rcp[:])

                # ---- o-proj + reduce-scatter for q-chunk(s) of j ----
                subchunks = [(j * 512, 512)] if j > 0 else [(0, 256),
                                                           (256, 256)]
                for (cs, cn) in subchunks:
                  for qt in range(cs // 128, (cs + cn) // 128):
                    ys = vp.tile([128, D], f32, name="ys", tag="ys",
                                 bufs=2)
                    for half in range(2):
                        op = opp.tile([128, 512], f32, name="op", tag="op",
                                      bufs=2)
                        for c in range(4):
                            nc.tensor.matmul(
                                op[:],
                                ctxT[c][:, qt * 128:(qt + 1) * 128],
                                woch[(c, half)][:],
                                start=(c == 0), stop=(c == 3),
                            )
                        nc.vector.tensor_copy(
                            ys[:, half * 512:(half + 1) * 512], op[:])
                    nc.sync.dma_start(rs_in[qt * 128:(qt + 1) * 128, :],
                                        ys[:])
                  if use_collective:
                    nc.gpsimd.collective_compute(
                        "ReduceScatter",
                        bass.mybir.AluOpType.add,
                        replica_groups=[[0, 1], [2, 3], [4, 5], [6, 7]],
                        ins=[rs_in[cs:cs + cn, :].opt()],
                        outs=[rs_out[cs // 2:(cs + cn) // 2, :].opt()],
                    )
                  else:
                    nc.sync.dma_start(rs_out[cs // 2:(cs + cn) // 2, :],
                                      rs_in[cs:cs + cn // 2, :])


            # ---- deferred tail: bias + relu on reduce-scattered shards ----
            tail_rows = [768, 896, 512, 640, 256, 384, 0, 128]
            for r0 in tail_rows:
                yt = vp.tile([128, D], f32, name="yt", tag="yt", bufs=2)
                nc.sync.dma_start(yt[:], rs_out[r0:r0 + 128, :])
                yo = vp.tile([128, D], f32, name="yo", tag="yo", bufs=2)
                nc.vector.tensor_add(yo[:], yt[:], bob_t[:])
                nc.scalar.activation(yo[:], yo[:], AF.Relu,
                                     bias=0.0, scale=1.0)
                nc.sync.dma_start(y_d[r0:r0 + 128, :], yo[:])

    lowp.__exit__(None, None, None)
    nc.compile()
    return nc


def _get_nc():
    if "nc" not in _cache:
        _cache["nc"] = _build()
    return _cache["nc"]


def kernel(x, Wq, bq, Wk, bk, Wv, bv, Wo, bo, trace=False):
    from concourse.bass_utils import run_bass_kernel_spmd

    x = np.asarray(x, np.float32)
    Wq, bq = np.asarray(Wq, np.float32), np.asarray(bq, np.float32)
    Wk, bk = np.asarray(Wk, np.float32), np.asarray(bk, np.float32)
    Wv, bv = np.asarray(Wv, np.float32), np.asarray(bv, np.float32)
    Wo, bo = np.asarray(Wo, np.float32), np.asarray(bo, np.float32)

    nc = _get_nc()
    in_maps = []
    for c in range(NCORES):
        b, hh = c // 2, c % 2
        sl = slice(hh * DC, (hh + 1) * DC)
        in_maps.append({
            "xT": np.ascontiguousarray(x[b].T).astype(ml_dtypes.bfloat16),
            "wq": np.ascontiguousarray(Wq[:, sl]).astype(ml_dtypes.bfloat16),
            "wk": np.ascontiguousarray(Wk[:, sl]).astype(ml_dtypes.bfloat16),
            "wv": np.ascontiguousarray(Wv[:, sl]).astype(ml_dtypes.bfloat16),
            "wo": np.ascontiguousarray(Wo[sl, :]),
            "bq": np.ascontiguousarray(bq[sl].reshape(4, 128).T),
            "bk": np.ascontiguousarray(bk[sl].reshape(4, 128).T),
            "bvb": np.ascontiguousarray(
                np.broadcast_to(bv[sl], (128, DC))),
            "bob": np.ascontiguousarray(np.broadcast_to(bo, (128, D))),
        })

    res = run_bass_kernel_spmd(nc, in_maps, core_ids=list(range(NCORES)),
                               trace=trace)
    _cache["last_result"] = res

    y = np.empty((B, S, D), np.float32)
    chunks = [(1536, 512), (1024, 512), (512, 512), (0, 256), (256, 256)]
    for c in range(NCORES):
        b, hh = c // 2, c % 2
        yp = res.results[c]["y"]
        for (s, n) in chunks:
            h = n // 2
            y[b, s + hh * h:s + hh * h + h, :] = yp[s // 2:s // 2 + h]
    return y
